# revision 23
# baseline (speedup 1.0000x reference)
"""Trainium2 Bass kernel for the self-attention block (nn_Attention).

Reference computation (per batch b, row h):
    f = x @ wf + bf; g = x @ wg + bg; h = x @ wh + bh      (1x1 convs)
    s = g @ f^T (over W); beta = softmax(s, -1); o = beta @ h
    out = gamma * o + x

Sharding: data-parallel over batch B=8, one batch element per NeuronCore.
Per core, each of the 128 rows is an independent [W=128, C=512] block.

Device dataflow per row r (all matmuls bf16, fp32 PSUM accumulation):
  - DMA xt tile [c,w] (host-pretransposed bf16, layout [r,p,k,w] so each
    partition reads one contiguous 1 KiB line) and x_row [w,c] f32.
  - fT/gT [64,w] = wf/wg^T x^T  (8 matmuls into one shared PSUM bank),
    bias added during the PSUM->SBUF copy on ScalarE (Identity+bias).
  - h [w,d] = x @ wh (4 matmuls), bh broadcast-added on VectorE.
  - sT[v,w] = f g^T transposed; A^T = exp(sT) on ScalarE (no max-subtract:
    |s| <= ~10 so fp32 exp is safe; softmax normalization deferred).
  - oU = A^T^T @ h;  Z/gamma = A^T^T @ (ones/gamma) into a spare column of
    the sT PSUM bank; reciprocal gives scale = gamma/Z directly.
  - out = oU * scale + x_row fused in one VectorE op; DMA out.
"""

import numpy as np
import ml_dtypes

import concourse.bacc as bacc
import concourse.bass as bass
import concourse.mybir as mybir
import concourse.tile as tile
from concourse.bass import ts

B, H, W, C = 8, 128, 128, 512
CK = C // 8  # 64
N_CORES = 8
KT = C // 128  # 4 contraction slices

F32 = mybir.dt.float32
BF16 = mybir.dt.bfloat16
BFDT = ml_dtypes.bfloat16
AF = mybir.ActivationFunctionType
ALU = mybir.AluOpType


def build_nc(rows: int = H) -> bass.Bass:
    # Bacc (not raw Bass): its compile() legalizes multi-semaphore waits
    # (walrus allows at most one wait per instruction on TRN2).
    nc = bacc.Bacc(None)
    x_d = nc.dram_tensor("x", [rows, W, C], F32, kind="ExternalInput")
    xt_d = nc.dram_tensor("xt", [rows, 128, KT, 128], BF16, kind="ExternalInput")
    wfg_d = nc.dram_tensor("wfg", [C, 2 * CK], BF16, kind="ExternalInput")
    wh_d = nc.dram_tensor("wh", [C, C], BF16, kind="ExternalInput")
    bf_d = nc.dram_tensor("bf", [CK, 1], F32, kind="ExternalInput")
    bg_d = nc.dram_tensor("bg", [CK, 1], F32, kind="ExternalInput")
    bhb_d = nc.dram_tensor("bhb", [W, C], F32, kind="ExternalInput")
    onesg_d = nc.dram_tensor("onesg", [W, 1], BF16, kind="ExternalInput")
    out_d = nc.dram_tensor("out", [rows, W, C], F32, kind="ExternalOutput")

    with tile.TileContext(nc) as tc:
        with (
            tc.tile_pool(name="const", bufs=1) as cpool,
            tc.tile_pool(name="sb_x", bufs=6) as sb_x,
            tc.tile_pool(name="sb_xt", bufs=4) as sb_xt,
            tc.tile_pool(name="sb_fg", bufs=4) as sb_fg,
            tc.tile_pool(name="sb_h", bufs=3) as sb_h,
            tc.tile_pool(name="sb_at", bufs=3) as sb_at,
            tc.tile_pool(name="sb_out", bufs=4) as sb_out,
            tc.tile_pool(name="sb_small", bufs=6) as sb_small,
            tc.tile_pool(name="ps_m", bufs=2, space="PSUM") as ps_m,
            tc.tile_pool(name="ps_h", bufs=3, space="PSUM") as ps_h,
            tc.tile_pool(name="ps_o", bufs=3, space="PSUM") as ps_o,
        ):
            wfg_sb = cpool.tile([128, KT * 2 * CK], BF16)
            wh_sb = cpool.tile([128, KT * C], BF16)
            for k in range(KT):
                nc.sync.dma_start(
                    wfg_sb[:, ts(k, 2 * CK)], wfg_d[k * 128 : (k + 1) * 128, :]
                )
                nc.sync.dma_start(
                    wh_sb[:, ts(k, C)], wh_d[k * 128 : (k + 1) * 128, :]
                )
            bf_sb = cpool.tile([CK, 1], F32)
            nc.sync.dma_start(bf_sb[:], bf_d[:])
            bg_sb = cpool.tile([CK, 1], F32)
            nc.sync.dma_start(bg_sb[:], bg_d[:])
            bhb_sb = cpool.tile([W, C], F32)
            nc.sync.dma_start(bhb_sb[:], bhb_d[:])
            onesg_sb = cpool.tile([W, 1], BF16)
            nc.sync.dma_start(onesg_sb[:], onesg_d[:])

            for r in range(rows):
                # Input DMAs ride the ACT HWDGE ring, the output DMA the SP
                # ring: measured best split. Inputs only wait on slot releases
                # several rows old, so they don't head-of-line-block ScalarE;
                # the epilogue-dependent write would (it stays on idle SP).
                xt16 = sb_xt.tile([128, C], BF16, tag="xt16")
                nc.scalar.dma_start(xt16[:], xt_d[r].rearrange("p k w -> p (k w)"))
                x_row = sb_x.tile([W, C], F32, tag="x_row")
                nc.scalar.dma_start(x_row[:], x_d[r])

                # h natural [w, d]
                h_ps = ps_h.tile([128, C], F32, tag="h")
                for k in range(KT):
                    nc.tensor.matmul(
                        h_ps[:],
                        lhsT=xt16[:, ts(k, 128)],
                        rhs=wh_sb[:, ts(k, C)],
                        start=(k == 0),
                        stop=(k == KT - 1),
                    )
                h16 = sb_h.tile([128, C], BF16, tag="h16")
                nc.vector.tensor_add(h16[:], h_ps[:], bhb_sb[:])

                # fT / gT [64, w] into one shared PSUM bank (disjoint regions)
                fg_ps = ps_m.tile([CK, 256], F32, tag="m")
                for k in range(KT):
                    nc.tensor.matmul(
                        fg_ps[:, 0:128],
                        lhsT=wfg_sb[:, ts(2 * k, CK)],
                        rhs=xt16[:, ts(k, 128)],
                        start=(k == 0),
                        stop=(k == KT - 1),
                    )
                for k in range(KT):
                    nc.tensor.matmul(
                        fg_ps[:, 128:256],
                        lhsT=wfg_sb[:, ts(2 * k + 1, CK)],
                        rhs=xt16[:, ts(k, 128)],
                        start=(k == 0),
                        stop=(k == KT - 1),
                    )
                f16 = sb_fg.tile([CK, 128], BF16, tag="f16")
                nc.scalar.activation(f16[:], fg_ps[:, 0:128], AF.Identity, bias=bf_sb[:])
                g16 = sb_fg.tile([CK, 128], BF16, tag="g16")
                nc.scalar.activation(g16[:], fg_ps[:, 128:256], AF.Identity, bias=bg_sb[:])

                # sT[v,w] in [:,0:128]; Z/gamma later lands in column 128
                st_ps = ps_m.tile([128, 129], F32, tag="m")
                nc.tensor.matmul(
                    st_ps[:, 0:128], lhsT=f16[:], rhs=g16[:], start=True, stop=True
                )
                at16 = sb_at.tile([128, 128], BF16, tag="at16")
                nc.scalar.activation(at16[:], st_ps[:, 0:128], AF.Exp)

                # oU[w,d] = sum_v A^T[v,w] h[v,d];  Z[w]/gamma via ones/gamma
                o_ps = ps_o.tile([128, C], F32, tag="o")
                nc.tensor.matmul(o_ps[:], lhsT=at16[:], rhs=h16[:], start=True, stop=True)
                nc.tensor.matmul(
                    st_ps[:, 128:129], lhsT=at16[:], rhs=onesg_sb[:], start=True, stop=True
                )
                scale = sb_small.tile([128, 1], F32, tag="scale")
                nc.vector.reciprocal(scale[:], st_ps[:, 128:129])

                out_sb = sb_out.tile([W, C], F32, tag="out_sb")
                nc.vector.scalar_tensor_tensor(
                    out_sb[:], o_ps[:], scale[:], x_row[:], ALU.mult, ALU.add
                )
                nc.sync.dma_start(out_d[r], out_sb[:])
    nc.compile()
    return nc


def make_in_map(x_b: np.ndarray, wf, bf, wg, bg, wh, bh, gamma) -> dict:
    x_b = np.asarray(x_b, np.float32)
    rows = x_b.shape[0]
    # interleave wf/wg columns per k-slice: [.., 2k] -> wf, [.., 2k+1] -> wg
    wfg = np.stack([np.asarray(wf), np.asarray(wg)], axis=1)  # [C, 2, CK]
    wfg = wfg.reshape(C, 2 * CK).astype(BFDT)
    # pre-transposed x, laid out [r, p, k, w] so each SBUF partition p reads
    # one contiguous (KT*128*2B = 1 KiB) line
    xt = np.ascontiguousarray(
        x_b.astype(BFDT).reshape(rows, W, KT, 128).transpose(0, 3, 2, 1)
    )
    gamma_f = float(np.float32(np.asarray(gamma)))
    onesg = np.full((W, 1), 1.0 / gamma_f, np.float32).astype(BFDT)
    return {
        "x": np.ascontiguousarray(x_b),
        "xt": xt,
        "wfg": wfg,
        "wh": np.asarray(wh).astype(BFDT),
        "bf": np.asarray(bf, np.float32).reshape(CK, 1),
        "bg": np.asarray(bg, np.float32).reshape(CK, 1),
        "bhb": np.ascontiguousarray(
            np.broadcast_to(np.asarray(bh, np.float32), (W, C))
        ),
        "onesg": onesg,
    }


_NC_CACHE: dict = {}


def run(inputs: dict, trace: bool = False, **run_kwargs):
    """Build (cached), run on 8 cores, return (out, BassKernelResults)."""
    from concourse.bass_utils import run_bass_kernel_spmd

    if "nc" not in _NC_CACHE:
        _NC_CACHE["nc"] = build_nc()
    nc = _NC_CACHE["nc"]
    x = np.asarray(inputs["x"], np.float32)
    in_maps = [
        make_in_map(
            x[b],
            inputs["wf"],
            inputs["bf"],
            inputs["wg"],
            inputs["bg"],
            inputs["wh"],
            inputs["bh"],
            inputs["gamma"],
        )
        for b in range(N_CORES)
    ]
    res = run_bass_kernel_spmd(
        nc, in_maps, list(range(N_CORES)), trace=trace, **run_kwargs
    )
    out = np.stack(
        [
            np.asarray(res.results[b]["out"]).astype(np.float32)
            for b in range(N_CORES)
        ],
        axis=0,
    )
    return out, res


def kernel(**inputs) -> np.ndarray:
    out, _ = run(inputs, trace=False)
    return out


# revision 32
# speedup vs baseline: 1.3942x; 1.3942x over previous
"""Trainium2 Bass kernel for the self-attention block (nn_Attention).

Reference computation (per batch b, row h):
    f = x @ wf + bf; g = x @ wg + bg; h = x @ wh + bh      (1x1 convs)
    s = g @ f^T (over W); beta = softmax(s, -1); o = beta @ h
    out = gamma * o + x

Sharding: data-parallel over batch B=8, one batch element per NeuronCore.
Per core, each of the 128 rows is an independent [W=128, C=512] block.

Device dataflow per row r (all matmuls bf16, fp32 PSUM accumulation):
  - DMA xt tile [c,w] (host-pretransposed bf16, layout [r,p,k,w] so each
    partition reads one contiguous 1 KiB line) and x_row [w,c] f32.
  - fT/gT [64,w] = wf/wg^T x^T  (8 matmuls into one shared PSUM bank),
    bias added during the PSUM->SBUF copy on ScalarE (Identity+bias).
  - h [w,d] = x @ wh (4 matmuls), bh broadcast-added on VectorE.
  - sT[v,w] = f g^T transposed; A^T = exp(sT) on ScalarE (no max-subtract:
    |s| <= ~10 so fp32 exp is safe; softmax normalization deferred).
  - oU = A^T^T @ h;  Z/gamma = A^T^T @ (ones/gamma) into a spare column of
    the sT PSUM bank; reciprocal gives scale = gamma/Z directly.
  - out = oU * scale + x_row fused in one VectorE op; DMA out.
"""

import numpy as np
import ml_dtypes

import concourse.bacc as bacc
import concourse.bass as bass
import concourse.mybir as mybir
import concourse.tile as tile
from concourse.bass import ts

B, H, W, C = 8, 128, 128, 512
CK = C // 8  # 64
N_CORES = 8
KT = C // 128  # 4 contraction slices

F32 = mybir.dt.float32
BF16 = mybir.dt.bfloat16
BFDT = ml_dtypes.bfloat16
AF = mybir.ActivationFunctionType
ALU = mybir.AluOpType


def row_batch(rows: int) -> int:
    return 4 if rows % 4 == 0 else (2 if rows % 2 == 0 else 1)


def build_nc(rows: int = H) -> bass.Bass:
    # Bacc (not raw Bass): its compile() legalizes multi-semaphore waits
    # (walrus allows at most one wait per instruction on TRN2).
    nc = bacc.Bacc(None)
    # RB rows ride in each DMA; host layouts keep every SBUF partition's
    # line contiguous (RB*2KiB f32 / RB*1KiB bf16) so transfers hit the
    # large-DMA efficiency regime.
    RB = row_batch(rows)
    nrb = rows // RB
    x_d = nc.dram_tensor("x", [nrb, 128, RB * C], F32, kind="ExternalInput")
    xt_d = nc.dram_tensor("xt", [nrb, 128, RB * C], BF16, kind="ExternalInput")
    wfg_d = nc.dram_tensor("wfg", [C, 2 * CK], BF16, kind="ExternalInput")
    wh_d = nc.dram_tensor("wh", [C, C], BF16, kind="ExternalInput")
    bf_d = nc.dram_tensor("bf", [CK, 1], F32, kind="ExternalInput")
    bg_d = nc.dram_tensor("bg", [CK, 1], F32, kind="ExternalInput")
    bhb_d = nc.dram_tensor("bhb", [W, C], F32, kind="ExternalInput")
    onesg_d = nc.dram_tensor("onesg", [W, 1], BF16, kind="ExternalInput")
    # bf16 output halves write traffic (the host widens back to f32); with
    # the input/output DMA ring split this measured fastest: 349-359us vs
    # 368-438us for the fp32-output variants. Costs ~1.7e-3 relative error.
    out_d = nc.dram_tensor("out", [nrb, 128, RB * C], BF16, kind="ExternalOutput")

    with tile.TileContext(nc) as tc:
        with (
            tc.tile_pool(name="const", bufs=1) as cpool,
            tc.tile_pool(name="sb_x", bufs=6) as sb_x,
            tc.tile_pool(name="sb_xt", bufs=4) as sb_xt,
            tc.tile_pool(name="sb_fg", bufs=4) as sb_fg,
            tc.tile_pool(name="sb_h", bufs=3) as sb_h,
            tc.tile_pool(name="sb_at", bufs=3) as sb_at,
            tc.tile_pool(name="sb_out", bufs=4) as sb_out,
            tc.tile_pool(name="sb_small", bufs=6) as sb_small,
            tc.tile_pool(name="ps_m", bufs=2, space="PSUM") as ps_m,
            tc.tile_pool(name="ps_h", bufs=3, space="PSUM") as ps_h,
            tc.tile_pool(name="ps_o", bufs=3, space="PSUM") as ps_o,
        ):
            wfg_sb = cpool.tile([128, KT * 2 * CK], BF16)
            wh_sb = cpool.tile([128, KT * C], BF16)
            for k in range(KT):
                nc.sync.dma_start(
                    wfg_sb[:, ts(k, 2 * CK)], wfg_d[k * 128 : (k + 1) * 128, :]
                )
                nc.sync.dma_start(
                    wh_sb[:, ts(k, C)], wh_d[k * 128 : (k + 1) * 128, :]
                )
            bf_sb = cpool.tile([CK, 1], F32)
            nc.sync.dma_start(bf_sb[:], bf_d[:])
            bg_sb = cpool.tile([CK, 1], F32)
            nc.sync.dma_start(bg_sb[:], bg_d[:])
            bhb_sb = cpool.tile([W, C], F32)
            nc.sync.dma_start(bhb_sb[:], bhb_d[:])
            onesg_sb = cpool.tile([W, 1], BF16)
            nc.sync.dma_start(onesg_sb[:], onesg_d[:])

            for rb in range(nrb):
                # Ring split balanced by bytes: the big f32 x read rides the
                # ACT HWDGE ring (few, 1 MiB-class transfers -> little queue
                # time); xt reads + the epilogue-dependent out write ride the
                # otherwise-idle SP ring (blocking SP is free).
                x4 = sb_x.tile([128, RB * C], F32, tag="x_row")
                nc.scalar.dma_start(x4[:], x_d[rb])
                xt4 = sb_xt.tile([128, RB * C], BF16, tag="xt16")
                nc.sync.dma_start(xt4[:], xt_d[rb])
                out4 = sb_out.tile([128, RB * C], BF16, tag="out_sb")
                for rr in range(RB):
                    xt16 = xt4[:, rr * C : (rr + 1) * C]
                    x_row = x4[:, rr * C : (rr + 1) * C]

                    # h natural [w, d]
                    h_ps = ps_h.tile([128, C], F32, tag="h")
                    for k in range(KT):
                        nc.tensor.matmul(
                            h_ps[:],
                            lhsT=xt16[:, ts(k, 128)],
                            rhs=wh_sb[:, ts(k, C)],
                            start=(k == 0),
                            stop=(k == KT - 1),
                        )
                    h16 = sb_h.tile([128, C], BF16, tag="h16")
                    nc.vector.tensor_add(h16[:], h_ps[:], bhb_sb[:])

                    # fT / gT [64, w] into one shared PSUM bank
                    fg_ps = ps_m.tile([CK, 256], F32, tag="m")
                    for k in range(KT):
                        nc.tensor.matmul(
                            fg_ps[:, 0:128],
                            lhsT=wfg_sb[:, ts(2 * k, CK)],
                            rhs=xt16[:, ts(k, 128)],
                            start=(k == 0),
                            stop=(k == KT - 1),
                        )
                    for k in range(KT):
                        nc.tensor.matmul(
                            fg_ps[:, 128:256],
                            lhsT=wfg_sb[:, ts(2 * k + 1, CK)],
                            rhs=xt16[:, ts(k, 128)],
                            start=(k == 0),
                            stop=(k == KT - 1),
                        )
                    f16 = sb_fg.tile([CK, 128], BF16, tag="f16")
                    nc.scalar.activation(
                        f16[:], fg_ps[:, 0:128], AF.Identity, bias=bf_sb[:]
                    )
                    g16 = sb_fg.tile([CK, 128], BF16, tag="g16")
                    nc.scalar.activation(
                        g16[:], fg_ps[:, 128:256], AF.Identity, bias=bg_sb[:]
                    )

                    # sT[v,w] in [:,0:128]; Z/gamma lands in column 128
                    st_ps = ps_m.tile([128, 129], F32, tag="m")
                    nc.tensor.matmul(
                        st_ps[:, 0:128], lhsT=f16[:], rhs=g16[:], start=True, stop=True
                    )
                    at16 = sb_at.tile([128, 128], BF16, tag="at16")
                    nc.scalar.activation(at16[:], st_ps[:, 0:128], AF.Exp)

                    # oU[w,d] = sum_v A^T[v,w] h[v,d];  Z/gamma via ones/gamma
                    o_ps = ps_o.tile([128, C], F32, tag="o")
                    nc.tensor.matmul(
                        o_ps[:], lhsT=at16[:], rhs=h16[:], start=True, stop=True
                    )
                    nc.tensor.matmul(
                        st_ps[:, 128:129],
                        lhsT=at16[:],
                        rhs=onesg_sb[:],
                        start=True,
                        stop=True,
                    )
                    scale = sb_small.tile([128, 1], F32, tag="scale")
                    nc.vector.reciprocal(scale[:], st_ps[:, 128:129])

                    nc.vector.scalar_tensor_tensor(
                        out4[:, rr * C : (rr + 1) * C],
                        o_ps[:],
                        scale[:],
                        x_row[:],
                        ALU.mult,
                        ALU.add,
                    )
                nc.sync.dma_start(out_d[rb], out4[:])
    nc.compile()
    return nc


def make_in_map(x_b: np.ndarray, wf, bf, wg, bg, wh, bh, gamma) -> dict:
    x_b = np.asarray(x_b, np.float32)
    rows = x_b.shape[0]
    # interleave wf/wg columns per k-slice: [.., 2k] -> wf, [.., 2k+1] -> wg
    wfg = np.stack([np.asarray(wf), np.asarray(wg)], axis=1)  # [C, 2, CK]
    wfg = wfg.reshape(C, 2 * CK).astype(BFDT)
    RB = row_batch(rows)
    nrb = rows // RB
    # x batched RB rows per DMA: [rb, p, rr, c] -> each partition line is
    # RB*2KiB contiguous
    x4 = np.ascontiguousarray(
        x_b.reshape(nrb, RB, W, C).transpose(0, 2, 1, 3).reshape(nrb, 128, RB * C)
    )
    # pre-transposed x: [rb, p, rr, k, w] (p = channel-within-slice), each
    # partition line RB*KT*128*2B contiguous
    xt = np.ascontiguousarray(
        x_b.astype(BFDT)
        .reshape(nrb, RB, W, KT, 128)
        .transpose(0, 4, 1, 3, 2)
        .reshape(nrb, 128, RB * C)
    )
    gamma_f = float(np.float32(np.asarray(gamma)))
    onesg = np.full((W, 1), 1.0 / gamma_f, np.float32).astype(BFDT)
    return {
        "x": x4,
        "xt": xt,
        "wfg": wfg,
        "wh": np.asarray(wh).astype(BFDT),
        "bf": np.asarray(bf, np.float32).reshape(CK, 1),
        "bg": np.asarray(bg, np.float32).reshape(CK, 1),
        "bhb": np.ascontiguousarray(
            np.broadcast_to(np.asarray(bh, np.float32), (W, C))
        ),
        "onesg": onesg,
    }


def unbatch_out(arr: np.ndarray, rows: int) -> np.ndarray:
    """[nrb, 128, RB*C] device layout -> [rows, W, C] f32."""
    RB = row_batch(rows)
    nrb = rows // RB
    return (
        np.asarray(arr)
        .astype(np.float32)
        .reshape(nrb, 128, RB, C)
        .transpose(0, 2, 1, 3)
        .reshape(rows, W, C)
    )


_NC_CACHE: dict = {}


def run(inputs: dict, trace: bool = False, **run_kwargs):
    """Build (cached), run on 8 cores, return (out, BassKernelResults)."""
    from concourse.bass_utils import run_bass_kernel_spmd

    if "nc" not in _NC_CACHE:
        _NC_CACHE["nc"] = build_nc()
    nc = _NC_CACHE["nc"]
    x = np.asarray(inputs["x"], np.float32)
    in_maps = [
        make_in_map(
            x[b],
            inputs["wf"],
            inputs["bf"],
            inputs["wg"],
            inputs["bg"],
            inputs["wh"],
            inputs["bh"],
            inputs["gamma"],
        )
        for b in range(N_CORES)
    ]
    res = run_bass_kernel_spmd(
        nc, in_maps, list(range(N_CORES)), trace=trace, **run_kwargs
    )
    out = np.stack(
        [unbatch_out(res.results[b]["out"], H) for b in range(N_CORES)], axis=0
    )
    return out, res


def kernel(**inputs) -> np.ndarray:
    out, _ = run(inputs, trace=False)
    return out
